# revision 14
# baseline (speedup 1.0000x reference)
"""Trainium2 Bass kernel for fake-quantized causal self-attention (8 NeuronCores).

Reference: per-tensor symmetric int8 fake-quant on x / Wq / Wk / Wv / y / Wo;
q/k/v/out projections via F.linear; causal softmax attention, 16 heads,
[B=2, T=2048, D=1024] fp32.

Sharding: core c handles batch b = c//4 and head-group hg = c%4 (4 heads =
256 of the 1024 head-dims).  Quantized projections run as integer-valued
bf16 matmuls (exact: operands are ints <= 127, products/sums fit fp32);
attention math runs in fp32.  Fake-quant scales resolve with a tiny
AllReduce(max); the quantized attention output is resharded with an int8
AllToAll so each core computes a 512-row slice of the output projection.
Host does only layout transforms (transpose/shard/concat).
"""

import sys
import numpy as np

sys.path.insert(0, "/opt/trn_rl_repo")

import concourse.bass as bass  # noqa: E402,F401
import concourse.bacc as bacc  # noqa: E402
import concourse.mybir as mybir  # noqa: E402
import concourse.tile as tile  # noqa: E402
import concourse.bass_isa as bass_isa  # noqa: E402
from concourse.bass_utils import run_bass_kernel_spmd  # noqa: E402

dt = mybir.dt
Alu = mybir.AluOpType
Act = mybir.ActivationFunctionType

B, T, D = 2, 2048, 1024
H, HD = 16, 64
NCORES = 8
GROUP = 4              # cores per batch group
HPC = H // GROUP       # heads per core (4)
DPC = HPC * HD         # head-dims per core (256)
TPC = T // GROUP       # output rows per core (512)
QMAX = 127.0
EPS = 1e-8
MAGIC = 12582912.0     # 1.5 * 2**23 : fp32 add/sub performs round-half-even
KC_X = D // 128        # contraction chunks over D (8)
NT = T // 128          # 128-row tiles over T (16)
QB = 512               # attention q-chunk width
NQB = T // QB          # 4
SC = float(HD) ** -0.5

_CACHE: dict = {}


def build():
    nc = bacc.Bacc("TRN2", target_bir_lowering=False, debug=False,
                   enable_asserts=False, num_devices=NCORES)

    xT = nc.dram_tensor("xT", [D, T], dt.float32, kind="ExternalInput").ap()
    wqT = nc.dram_tensor("wqT", [D, DPC], dt.float32, kind="ExternalInput").ap()
    wkT = nc.dram_tensor("wkT", [D, DPC], dt.float32, kind="ExternalInput").ap()
    wvT = nc.dram_tensor("wvT", [D, DPC], dt.float32, kind="ExternalInput").ap()
    woT = nc.dram_tensor("woT", [D, D], dt.float32, kind="ExternalInput").ap()
    bq_d = nc.dram_tensor("bq", [DPC], dt.float32, kind="ExternalInput").ap()
    bk_d = nc.dram_tensor("bk", [DPC], dt.float32, kind="ExternalInput").ap()
    bv_d = nc.dram_tensor("bv", [DPC], dt.float32, kind="ExternalInput").ap()
    bo_d = nc.dram_tensor("bo", [D], dt.float32, kind="ExternalInput").ap()
    masks_d = nc.dram_tensor("masks", [4, 128, 2 * QB], dt.float32,
                             kind="ExternalInput").ap()
    toff_d = nc.dram_tensor("toff", [1, 1], dt.uint32, kind="ExternalInput").ap()
    out_d = nc.dram_tensor("out", [TPC, D], dt.float32, kind="ExternalOutput").ap()

    cc1_in = nc.dram_tensor("cc1_in", [1, 8], dt.float32)
    cc1_out = nc.dram_tensor("cc1_out", [1, 8], dt.float32, addr_space="Shared")
    cc2_in = nc.dram_tensor("cc2_in", [1, 8], dt.float32)
    cc2_out = nc.dram_tensor("cc2_out", [1, 8], dt.float32, addr_space="Shared")
    ag_in = nc.dram_tensor("ag_in", [DPC, T], dt.int8)
    ag_out = nc.dram_tensor("ag_out", [GROUP * DPC, T], dt.int8)

    groups_all = [list(range(NCORES))]
    groups_b = [[0, 1, 2, 3], [4, 5, 6, 7]]

    with tile.TileContext(nc, trace_sim=False) as tc:
        with (
            tc.tile_pool(name="per0", bufs=1) as per0,   # scalars/biases
            tc.tile_pool(name="wkp", bufs=2) as wk,      # small working tiles
            tc.tile_pool(name="ptp", bufs=2) as ptp,     # attention PT tiles
            tc.tile_pool(name="psp", bufs=2, space="PSUM") as psp,
            tc.tile_pool(name="psy", bufs=1, space="PSUM") as psy,
        ):
            def absmax_of(tiles, tag):
                m = wk.tile([128, 1], dt.float32, tag=f"am_{tag}", name=f"am_{tag}")
                for i, s in enumerate(tiles):
                    if i == 0:
                        nc.vector.tensor_reduce(
                            m[:], s[:], axis=mybir.AxisListType.X,
                            op=Alu.max, apply_absolute_value=True)
                    else:
                        t = wk.tile([128, 1], dt.float32, tag="am_t", name="am_t")
                        nc.vector.tensor_reduce(
                            t[:], s[:], axis=mybir.AxisListType.X,
                            op=Alu.max, apply_absolute_value=True)
                        nc.vector.tensor_tensor(m[:], m[:], t[:], op=Alu.max)
                ma = wk.tile([128, 1], dt.float32, tag=f"aa_{tag}", name=f"aa_{tag}")
                nc.gpsimd.partition_all_reduce(
                    ma[:], m[:], channels=128, reduce_op=bass_isa.ReduceOp.absmax)
                return ma

            def quantize(src_tiles, inv_scale_bc, out_dtype, tag, pool):
                outs = []
                for i, s in enumerate(src_tiles):
                    shape = list(s.shape)
                    t = wk.tile([128, T], dt.float32, tag="qtmp", name="qtmp")
                    tv = t[:, 0:s.shape[-1]] if len(shape) == 2 else t[:]
                    nc.scalar.activation(tv, s[:], Act.Copy, scale=inv_scale_bc[:])
                    nc.vector.tensor_scalar(tv, tv, MAGIC, MAGIC,
                                            op0=Alu.add, op1=Alu.subtract)
                    o = pool.tile(shape, out_dtype, tag=f"{tag}{i}", name=f"{tag}{i}")
                    nc.vector.tensor_scalar(o[:], tv, QMAX, -QMAX - 1.0,
                                            op0=Alu.min, op1=Alu.max)
                    outs.append(o)
                return outs

            def bc128(src_ap, tag):
                t = per0.tile([128, 1], dt.float32, tag=tag, name=tag)
                nc.gpsimd.partition_broadcast(t[:], src_ap)
                return t

            # biases (small, live throughout)
            btiles = {}
            for bname, src in (("bq", bq_d), ("bk", bk_d)):
                bt = per0.tile([128, 2], dt.float32, tag=f"{bname}t", name=f"{bname}t")
                nc.sync.dma_start(out=bt[:], in_=src.rearrange("(a p) -> p a", p=128))
                btiles[bname] = bt
            bv_bc = per0.tile([128, DPC], dt.float32, tag="bv_bc", name="bv_bc")
            nc.sync.dma_start(out=bv_bc[:], in_=bv_d[None, :].partition_broadcast(128))
            bo_bc = per0.tile([128, D], dt.float32, tag="bo_bc", name="bo_bc")
            nc.sync.dma_start(out=bo_bc[:], in_=bo_d[None, :].partition_broadcast(128))

            with tc.tile_pool(name="woqp", bufs=1) as woqp, \
                 tc.tile_pool(name="xqp", bufs=1) as xqp:
                # ---------------- Phase 0-2: load, scales, quantize ------
                with tc.tile_pool(name="stage", bufs=1) as stage:
                    xf = []
                    for i in range(KC_X):
                        t = stage.tile([128, T], dt.float32, tag=f"xf{i}",
                                       name=f"xf{i}")
                        nc.sync.dma_start(out=t[:], in_=xT[i * 128:(i + 1) * 128, :])
                        xf.append(t)
                    wfs = {}
                    for wname, src, width in (("wq", wqT, DPC), ("wk", wkT, DPC),
                                              ("wv", wvT, DPC)):
                        tiles = []
                        for i in range(KC_X):
                            t = stage.tile([128, width], dt.float32,
                                           tag=f"{wname}f{i}", name=f"{wname}f{i}")
                            nc.sync.dma_start(out=t[:],
                                              in_=src[i * 128:(i + 1) * 128, :])
                            tiles.append(t)
                        wfs[wname] = tiles

                    m_x = absmax_of(xf, "x")
                    m_wq = absmax_of(wfs["wq"], "wq")
                    m_wk = absmax_of(wfs["wk"], "wk")
                    m_wv = absmax_of(wfs["wv"], "wv")
                    # wo: streamed (too big to stage alongside x)
                    m_wo_p = wk.tile([128, 1], dt.float32, tag="am_wo",
                                     name="am_wo")
                    for i in range(KC_X):
                        wot = wk.tile([128, D], dt.float32, tag="wostg",
                                      name="wostg")
                        nc.sync.dma_start(out=wot[:],
                                          in_=woT[i * 128:(i + 1) * 128, :])
                        if i == 0:
                            nc.vector.tensor_reduce(
                                m_wo_p[:], wot[:], axis=mybir.AxisListType.X,
                                op=Alu.max, apply_absolute_value=True)
                        else:
                            t2 = wk.tile([128, 1], dt.float32, tag="am_t",
                                         name="am_t")
                            nc.vector.tensor_reduce(
                                t2[:], wot[:], axis=mybir.AxisListType.X,
                                op=Alu.max, apply_absolute_value=True)
                            nc.vector.tensor_tensor(m_wo_p[:], m_wo_p[:], t2[:],
                                                    op=Alu.max)
                    m_wo = wk.tile([128, 1], dt.float32, tag="aa_wo", name="aa_wo")
                    nc.gpsimd.partition_all_reduce(
                        m_wo[:], m_wo_p[:], channels=128,
                        reduce_op=bass_isa.ReduceOp.absmax)

                    v8 = per0.tile([1, 8], dt.float32, tag="v8", name="v8")
                    nc.vector.memset(v8[:], 0.0)
                    for j, m in enumerate((m_x, m_wq, m_wk, m_wv, m_wo)):
                        nc.vector.tensor_copy(v8[:, j:j + 1], m[0:1, :])
                    nc.sync.dma_start(out=cc1_in.ap(), in_=v8[:])
                    nc.gpsimd.collective_compute(
                        "AllReduce", Alu.max, replica_groups=groups_all,
                        ins=[cc1_in.ap()], outs=[cc1_out.ap()])
                    g8 = per0.tile([1, 8], dt.float32, tag="g8", name="g8")
                    nc.sync.dma_start(out=g8[:], in_=cc1_out.ap())

                    s8 = per0.tile([1, 8], dt.float32, tag="s8", name="s8")
                    nc.vector.tensor_scalar(s8[:], g8[:], 1.0 / QMAX, EPS,
                                            op0=Alu.mult, op1=Alu.max)
                    inv8 = per0.tile([1, 8], dt.float32, tag="inv8", name="inv8")
                    nc.vector.reciprocal(inv8[:], s8[:])

                    inv_x = bc128(inv8[:, 0:1], "inv_x")
                    inv_q = bc128(inv8[:, 1:2], "inv_q")
                    inv_k = bc128(inv8[:, 2:3], "inv_k")
                    inv_v = bc128(inv8[:, 3:4], "inv_v")
                    inv_o = bc128(inv8[:, 4:5], "inv_o")

                    prod = per0.tile([1, 3], dt.float32, tag="prod", name="prod")
                    for j in range(3):
                        nc.vector.tensor_tensor(prod[:, j:j + 1], s8[:, 0:1],
                                                s8[:, j + 1:j + 2], op=Alu.mult)
                    sxwq = bc128(prod[:, 0:1], "sxwq")
                    sxwk = bc128(prod[:, 1:2], "sxwk")
                    sxwv = bc128(prod[:, 2:3], "sxwv")

                    xq = quantize(xf, inv_x, dt.bfloat16, "xq", xqp)
                    wqq = quantize(wfs["wq"], inv_q, dt.bfloat16, "wqq", xqp)
                    wkq = quantize(wfs["wk"], inv_k, dt.bfloat16, "wkq", xqp)
                    wvq = quantize(wfs["wv"], inv_v, dt.bfloat16, "wvq", xqp)

                # wo re-streamed + quantized (stage scope now closed)
                woq = []
                for i in range(KC_X):
                    wot = wk.tile([128, D], dt.float32, tag="wostg", name="wostg")
                    nc.sync.dma_start(out=wot[:], in_=woT[i * 128:(i + 1) * 128, :])
                    woq += quantize([wot], inv_o, dt.bfloat16, f"woq{i}_", woqp)

                # ---------------- Phase 3: q/k/v projections -------------
                with tc.tile_pool(name="qkvp", bufs=1) as qkvp:
                    qT, kT = [], []
                    for pname, wql, bt, sxw, dst in (
                            ("q", wqq, btiles["bq"], sxwq, qT),
                            ("k", wkq, btiles["bk"], sxwk, kT)):
                        for p in range(2):
                            qt_ = qkvp.tile([128, T], dt.float32,
                                            tag=f"{pname}T{p}", name=f"{pname}T{p}")
                            for tc_ in range(NQB):
                                ps = psp.tile([128, QB], dt.float32, tag="proj",
                                              name="proj_ps")
                                for kc in range(KC_X):
                                    nc.tensor.matmul(
                                        ps[:], wql[kc][:, p * 128:(p + 1) * 128],
                                        xq[kc][:, tc_ * QB:(tc_ + 1) * QB],
                                        start=(kc == 0), stop=(kc == KC_X - 1))
                                nc.vector.tensor_scalar(
                                    qt_[:, tc_ * QB:(tc_ + 1) * QB], ps[:],
                                    sxw[:], bt[:, p:p + 1],
                                    op0=Alu.mult, op1=Alu.add)
                            dst.append(qt_)

                    v_sb = []
                    for tt in range(NT):
                        vt = qkvp.tile([128, HPC, HD + 1], dt.float32,
                                       tag=f"v{tt}", name=f"v{tt}")
                        nc.vector.memset(vt[:], 1.0)
                        ps = psp.tile([128, DPC], dt.float32, tag="proj",
                                      name="projv_ps")
                        for kc in range(KC_X):
                            nc.tensor.matmul(ps[:],
                                             xq[kc][:, tt * 128:(tt + 1) * 128],
                                             wvq[kc][:], start=(kc == 0),
                                             stop=(kc == KC_X - 1))
                        nc.vector.scalar_tensor_tensor(
                            vt[:, :, 0:HD],
                            ps[:].rearrange("p (h d) -> p h d", d=HD),
                            sxwv[:], bv_bc[:].rearrange("p (h d) -> p h d", d=HD),
                            op0=Alu.mult, op1=Alu.add)
                        v_sb.append(vt)

                    # ------------- Phase 4: attention ---------------------
                    with tc.tile_pool(name="attp", bufs=1) as attp:
                        masks = []
                        for r in range(4):
                            mt = attp.tile([128, 2 * QB], dt.float32,
                                           tag=f"mask{r}", name=f"mask{r}")
                            nc.sync.dma_start(out=mt[:], in_=masks_d[r])
                            masks.append(mt)

                        yT = [attp.tile([128, T], dt.float32, tag=f"yT{p}",
                                        name=f"yT{p}") for p in range(2)]
                        ymax = per0.tile([64, 1], dt.float32, tag="ymax",
                                         name="ymax")
                        nc.vector.memset(ymax[:], 0.0)

                        for p in range(2):          # head pair (dims tile)
                            for qb in range(NQB):
                                q0 = qb * QB
                                kmax = 4 * qb + 4
                                psA = psy.tile([HD + 1, QB], dt.float32,
                                               tag="psYA", name="psYA")
                                psB = psy.tile([HD + 1, QB], dt.float32,
                                               tag="psYB", name="psYB")
                                for kc in range(kmax):
                                    psS = psp.tile([128, 2 * QB], dt.float32,
                                                   tag="psS", name="psS")
                                    k0 = kc * 128
                                    nc.tensor.matmul(
                                        psS[:, 0:QB], kT[p][0:64, k0:k0 + 128],
                                        qT[p][0:64, q0:q0 + QB],
                                        start=True, stop=True,
                                        tile_position=(0, 0))
                                    nc.tensor.matmul(
                                        psS[:, QB:2 * QB],
                                        kT[p][64:128, k0:k0 + 128],
                                        qT[p][64:128, q0:q0 + QB],
                                        start=True, stop=True,
                                        tile_position=(64, 0))
                                    pt_ = ptp.tile([128, 2 * QB], dt.float32,
                                                   tag="PT", name="PT")
                                    nc.scalar.activation(pt_[:], psS[:], Act.Exp,
                                                         scale=SC)
                                    if kc >= 4 * qb:
                                        nc.vector.tensor_tensor(
                                            pt_[:], pt_[:],
                                            masks[kc - 4 * qb][:], op=Alu.mult)
                                    nc.tensor.matmul(
                                        psA[:], v_sb[kc][:, 2 * p, :],
                                        pt_[:, 0:QB],
                                        start=(kc == 0), stop=(kc == kmax - 1))
                                    nc.tensor.matmul(
                                        psB[:], v_sb[kc][:, 2 * p + 1, :],
                                        pt_[:, QB:2 * QB],
                                        start=(kc == 0), stop=(kc == kmax - 1))
                                for hl, psY in ((0, psA), (1, psB)):
                                    r1 = wk.tile([1, QB], dt.float32, tag="r1",
                                                 name="r1")
                                    nc.vector.reciprocal(r1[:], psY[64:65, :])
                                    rb = wk.tile([64, QB], dt.float32, tag="rb",
                                                 name="rb")
                                    nc.gpsimd.partition_broadcast(rb[:], r1[:])
                                    ysl = yT[p][hl * 64:(hl + 1) * 64,
                                                q0:q0 + QB]
                                    mt = wk.tile([64, 1], dt.float32, tag="ymt",
                                                 name="ymt")
                                    nc.vector.tensor_tensor(
                                        ysl, psY[0:64, :], rb[:], op=Alu.mult)
                                    nc.vector.tensor_reduce(
                                        mt[:], ysl, axis=mybir.AxisListType.X,
                                        op=Alu.max, apply_absolute_value=True)
                                    nc.vector.tensor_tensor(ymax[:], ymax[:],
                                                            mt[:], op=Alu.max)

                        # ------------- Phase 5: y scale (AllReduce) -------
                        yma = wk.tile([64, 1], dt.float32, tag="yma", name="yma")
                        nc.gpsimd.partition_all_reduce(
                            yma[:], ymax[:], channels=64,
                            reduce_op=bass_isa.ReduceOp.max)
                        v2 = per0.tile([1, 8], dt.float32, tag="v2", name="v2")
                        nc.vector.memset(v2[:], 0.0)
                        nc.vector.tensor_copy(v2[:, 0:1], yma[0:1, :])
                        nc.sync.dma_start(out=cc2_in.ap(), in_=v2[:])
                        nc.gpsimd.collective_compute(
                            "AllReduce", Alu.max, replica_groups=groups_all,
                            ins=[cc2_in.ap()], outs=[cc2_out.ap()])
                        g2 = per0.tile([1, 8], dt.float32, tag="g2", name="g2")
                        nc.sync.dma_start(out=g2[:], in_=cc2_out.ap())
                        sy = per0.tile([1, 1], dt.float32, tag="sy", name="sy")
                        nc.vector.tensor_scalar(sy[:], g2[:, 0:1], 1.0 / QMAX,
                                                EPS, op0=Alu.mult, op1=Alu.max)
                        inv_sy1 = per0.tile([1, 1], dt.float32, tag="inv_sy1",
                                            name="inv_sy1")
                        nc.vector.reciprocal(inv_sy1[:], sy[:])
                        inv_sy = bc128(inv_sy1[:], "inv_sy")
                        sywo1 = per0.tile([1, 1], dt.float32, tag="sywo1",
                                          name="sywo1")
                        nc.vector.tensor_tensor(sywo1[:], sy[:], s8[:, 4:5],
                                                op=Alu.mult)
                        sywo = bc128(sywo1[:], "sywo")

                        # ------------- Phase 6: quantize y, AllGather -----
                        with tc.tile_pool(name="latep", bufs=1) as latep:
                            yq = quantize(yT, inv_sy, dt.int8, "yq", latep)
                            for p in range(2):
                                nc.sync.dma_start(
                                    out=ag_in.ap()[p * 128:(p + 1) * 128, :],
                                    in_=yq[p][:])
                            nc.gpsimd.collective_compute(
                                "AllGather", Alu.bypass, replica_groups=groups_b,
                                ins=[ag_in.ap()], outs=[ag_out.ap()])
                            treg = nc.alloc_registers()
                            nc.regs_load(treg, toff_d[0:1, 0:1])
                            toff = nc.snap(treg, donate=True, min_val=0,
                                           max_val=T - QB)
                            yg = []
                            for i in range(KC_X):
                                gi = wk.tile([128, QB], dt.int8, tag="ygi",
                                             name="ygi")
                                nc.sync.dma_start(
                                    out=gi[:],
                                    in_=ag_out.ap()[i * 128:(i + 1) * 128,
                                                    bass.ds(toff, QB)])
                                gb = latep.tile([128, QB], dt.bfloat16,
                                                tag=f"ygb{i}", name=f"ygb{i}")
                                nc.vector.tensor_copy(gb[:], gi[:])
                                yg.append(gb)

                            # --------- Phase 7: output projection ---------
                            for tt in range(TPC // 128):
                                ot = latep.tile([128, D], dt.float32, tag="ot",
                                                name="ot")
                                for oc in range(2):
                                    ps = psp.tile([128, QB], dt.float32,
                                                  tag="proj", name="proj_o")
                                    for kc in range(KC_X):
                                        nc.tensor.matmul(
                                            ps[:],
                                            yg[kc][:, tt * 128:(tt + 1) * 128],
                                            woq[kc][:, oc * QB:(oc + 1) * QB],
                                            start=(kc == 0),
                                            stop=(kc == KC_X - 1))
                                    nc.vector.scalar_tensor_tensor(
                                        ot[:, oc * QB:(oc + 1) * QB], ps[:],
                                        sywo[:], bo_bc[:, oc * QB:(oc + 1) * QB],
                                        op0=Alu.mult, op1=Alu.add)
                                nc.sync.dma_start(
                                    out=out_d[tt * 128:(tt + 1) * 128, :],
                                    in_=ot[:])

    nc.compile()
    return nc


def _masks_np():
    """masks[r][k, q] = 1 if q >= k + 128*r else 0, duplicated for both heads."""
    m = np.zeros((4, 128, 2 * QB), np.float32)
    q = np.arange(QB)[None, :]
    k = np.arange(128)[:, None]
    for r in range(4):
        mm = (q >= k + 128 * r).astype(np.float32)
        m[r, :, 0:QB] = mm
        m[r, :, QB:2 * QB] = mm
    return m


def make_in_maps(x, Wq, bq, Wk, bk, Wv, bv, Wo, bo):
    masks = _masks_np()
    woT = np.ascontiguousarray(Wo.T)
    xTs = [np.ascontiguousarray(x[b].T) for b in range(B)]
    wT = {}
    for name, W in (("q", Wq), ("k", Wk), ("v", Wv)):
        wT[name] = [np.ascontiguousarray(W[g * DPC:(g + 1) * DPC, :].T)
                    for g in range(GROUP)]
    in_maps = []
    for c in range(NCORES):
        b, hg = c // GROUP, c % GROUP
        in_maps.append({
            "xT": xTs[b],
            "wqT": wT["q"][hg], "wkT": wT["k"][hg], "wvT": wT["v"][hg],
            "woT": woT,
            "bq": np.ascontiguousarray(bq[hg * DPC:(hg + 1) * DPC]),
            "bk": np.ascontiguousarray(bk[hg * DPC:(hg + 1) * DPC]),
            "bv": np.ascontiguousarray(bv[hg * DPC:(hg + 1) * DPC]),
            "bo": bo,
            "masks": masks,
            "toff": np.array([[hg * QB]], dtype=np.uint32),
        })
    return in_maps


def assemble(results):
    out = np.empty((B, T, D), np.float32)
    for c in range(NCORES):
        b, hg = c // GROUP, c % GROUP
        out[b, hg * TPC:(hg + 1) * TPC, :] = results[c]["out"]
    return out


def kernel(x, Wq, bq, Wk, bk, Wv, bv, Wo, bo):
    if "nc" not in _CACHE:
        _CACHE["nc"] = build()
    nc = _CACHE["nc"]
    in_maps = make_in_maps(x, Wq, bq, Wk, bk, Wv, bv, Wo, bo)
    res = run_bass_kernel_spmd(nc, in_maps, list(range(NCORES)))
    return assemble(res.results)


# revision 19
# speedup vs baseline: 1.2593x; 1.2593x over previous
"""Trainium2 Bass kernel for fake-quantized causal self-attention (8 NeuronCores).

Reference: per-tensor symmetric int8 fake-quant on x / Wq / Wk / Wv / y / Wo;
q/k/v/out projections via F.linear; causal softmax attention, 16 heads,
[B=2, T=2048, D=1024] fp32.

Sharding: core c handles batch b = c//4 and head-group hg = c%4 (4 heads =
256 of the 1024 head-dims).  Quantized projections run as integer-valued
bf16 matmuls (exact: operands are ints <= 127, products/sums fit fp32);
attention math runs in fp32.  Fake-quant scales resolve with a tiny
AllReduce(max); the quantized attention output is resharded with an int8
AllToAll so each core computes a 512-row slice of the output projection.
Host does only layout transforms (transpose/shard/concat).
"""

import sys
import numpy as np

sys.path.insert(0, "/opt/trn_rl_repo")

import concourse.bass as bass  # noqa: E402,F401
import concourse.bacc as bacc  # noqa: E402
import concourse.mybir as mybir  # noqa: E402
import concourse.tile as tile  # noqa: E402
import concourse.bass_isa as bass_isa  # noqa: E402
from concourse.bass_utils import run_bass_kernel_spmd  # noqa: E402

dt = mybir.dt
Alu = mybir.AluOpType
Act = mybir.ActivationFunctionType

B, T, D = 2, 2048, 1024
H, HD = 16, 64
NCORES = 8
GROUP = 4              # cores per batch group
HPC = H // GROUP       # heads per core (4)
DPC = HPC * HD         # head-dims per core (256)
TPC = T // GROUP       # output rows per core (512)
QMAX = 127.0
EPS = 1e-8
MAGIC = 12582912.0     # 1.5 * 2**23 : fp32 add/sub performs round-half-even
KC_X = D // 128        # contraction chunks over D (8)
NT = T // 128          # 128-row tiles over T (16)
QB = 512               # attention q-chunk width
NQB = T // QB          # 4
SC = float(HD) ** -0.5
F32R = mybir.dt.float32r

_CACHE: dict = {}


def build():
    nc = bacc.Bacc("TRN2", target_bir_lowering=False, debug=False,
                   enable_asserts=False, num_devices=NCORES)

    xT = nc.dram_tensor("xT", [D, T], dt.float32, kind="ExternalInput").ap()
    wqT = nc.dram_tensor("wqT", [D, DPC], dt.float32, kind="ExternalInput").ap()
    wkT = nc.dram_tensor("wkT", [D, DPC], dt.float32, kind="ExternalInput").ap()
    wvT = nc.dram_tensor("wvT", [D, DPC], dt.float32, kind="ExternalInput").ap()
    woT = nc.dram_tensor("woT", [D, D], dt.float32, kind="ExternalInput").ap()
    bq_d = nc.dram_tensor("bq", [DPC], dt.float32, kind="ExternalInput").ap()
    bk_d = nc.dram_tensor("bk", [DPC], dt.float32, kind="ExternalInput").ap()
    bv_d = nc.dram_tensor("bv", [DPC], dt.float32, kind="ExternalInput").ap()
    bo_d = nc.dram_tensor("bo", [D], dt.float32, kind="ExternalInput").ap()
    masks_d = nc.dram_tensor("masks", [4, 128, 2 * QB], dt.float32,
                             kind="ExternalInput").ap()
    toff_d = nc.dram_tensor("toff", [1, 1], dt.uint32, kind="ExternalInput").ap()
    out_d = nc.dram_tensor("out", [TPC, D], dt.float32, kind="ExternalOutput").ap()

    cc1_in = nc.dram_tensor("cc1_in", [1, 8], dt.float32)
    cc1_out = nc.dram_tensor("cc1_out", [1, 8], dt.float32, addr_space="Shared")
    cc2_in = nc.dram_tensor("cc2_in", [1, 8], dt.float32)
    cc2_out = nc.dram_tensor("cc2_out", [1, 8], dt.float32, addr_space="Shared")
    ag_in = nc.dram_tensor("ag_in", [DPC, T], dt.int8)
    ag_out = nc.dram_tensor("ag_out", [GROUP * DPC, T], dt.int8)

    groups_all = [list(range(NCORES))]
    groups_b = [[0, 1, 2, 3], [4, 5, 6, 7]]

    with tile.TileContext(nc, trace_sim=False) as tc:
        with (
            tc.tile_pool(name="per0", bufs=1) as per0,   # scalars/biases
            tc.tile_pool(name="wkp", bufs=2) as wk,      # small working tiles
            tc.tile_pool(name="ptp", bufs=2) as ptp,     # attention PT tiles
            tc.tile_pool(name="psp", bufs=2, space="PSUM") as psp,
            tc.tile_pool(name="psy", bufs=1, space="PSUM") as psy,
        ):
            def absmax_of(tiles, tag):
                m = wk.tile([128, 1], dt.float32, tag=f"am_{tag}", name=f"am_{tag}")
                for i, s in enumerate(tiles):
                    if i == 0:
                        nc.vector.tensor_reduce(
                            m[:], s[:], axis=mybir.AxisListType.X,
                            op=Alu.max, apply_absolute_value=True)
                    else:
                        t = wk.tile([128, 1], dt.float32, tag="am_t", name="am_t")
                        nc.vector.tensor_reduce(
                            t[:], s[:], axis=mybir.AxisListType.X,
                            op=Alu.max, apply_absolute_value=True)
                        nc.vector.tensor_tensor(m[:], m[:], t[:], op=Alu.max)
                ma = wk.tile([128, 1], dt.float32, tag=f"aa_{tag}", name=f"aa_{tag}")
                nc.gpsimd.partition_all_reduce(
                    ma[:], m[:], channels=128, reduce_op=bass_isa.ReduceOp.absmax)
                return ma

            def quantize(src_tiles, inv_scale_bc, out_dtype, tag, pool):
                outs = []
                for i, s in enumerate(src_tiles):
                    shape = list(s.shape)
                    t = wk.tile([128, T], dt.float32, tag="qtmp", name="qtmp")
                    tv = t[:, 0:s.shape[-1]] if len(shape) == 2 else t[:]
                    nc.scalar.activation(tv, s[:], Act.Copy, scale=inv_scale_bc[:])
                    nc.vector.tensor_scalar(tv, tv, MAGIC, MAGIC,
                                            op0=Alu.add, op1=Alu.subtract)
                    o = pool.tile(shape, out_dtype, tag=f"{tag}{i}", name=f"{tag}{i}")
                    nc.vector.tensor_scalar(o[:], tv, QMAX, -QMAX - 1.0,
                                            op0=Alu.min, op1=Alu.max)
                    outs.append(o)
                return outs

            def bc128(src_ap, tag):
                t = per0.tile([128, 1], dt.float32, tag=tag, name=tag)
                nc.gpsimd.partition_broadcast(t[:], src_ap)
                return t

            # biases (small, live throughout)
            btiles = {}
            for bname, src in (("bq", bq_d), ("bk", bk_d)):
                bt = per0.tile([128, 2], dt.float32, tag=f"{bname}t", name=f"{bname}t")
                nc.sync.dma_start(out=bt[:], in_=src.rearrange("(a p) -> p a", p=128))
                btiles[bname] = bt
            bv_bc = per0.tile([128, DPC], dt.float32, tag="bv_bc", name="bv_bc")
            nc.sync.dma_start(out=bv_bc[:], in_=bv_d[None, :].partition_broadcast(128))
            bo_bc = per0.tile([128, D], dt.float32, tag="bo_bc", name="bo_bc")
            nc.sync.dma_start(out=bo_bc[:], in_=bo_d[None, :].partition_broadcast(128))

            with tc.tile_pool(name="woqp", bufs=1) as woqp, \
                 tc.tile_pool(name="xqp", bufs=1) as xqp:
                # ---------------- Phase 0-2: load, scales, quantize ------
                with tc.tile_pool(name="stage", bufs=1) as stage:
                    xf = []
                    for i in range(KC_X):
                        t = stage.tile([128, T], dt.float32, tag=f"xf{i}",
                                       name=f"xf{i}")
                        nc.sync.dma_start(out=t[:], in_=xT[i * 128:(i + 1) * 128, :])
                        xf.append(t)
                    wfs = {}
                    for wname, src, width in (("wq", wqT, DPC), ("wk", wkT, DPC),
                                              ("wv", wvT, DPC)):
                        tiles = []
                        for i in range(KC_X):
                            t = stage.tile([128, width], dt.float32,
                                           tag=f"{wname}f{i}", name=f"{wname}f{i}")
                            nc.sync.dma_start(out=t[:],
                                              in_=src[i * 128:(i + 1) * 128, :])
                            tiles.append(t)
                        wfs[wname] = tiles

                    m_x = absmax_of(xf, "x")
                    m_wq = absmax_of(wfs["wq"], "wq")
                    m_wk = absmax_of(wfs["wk"], "wk")
                    m_wv = absmax_of(wfs["wv"], "wv")
                    # wo: streamed (too big to stage alongside x)
                    m_wo_p = wk.tile([128, 1], dt.float32, tag="am_wo",
                                     name="am_wo")
                    for i in range(KC_X):
                        wot = wk.tile([128, D], dt.float32, tag="wostg",
                                      name="wostg")
                        nc.sync.dma_start(out=wot[:],
                                          in_=woT[i * 128:(i + 1) * 128, :])
                        if i == 0:
                            nc.vector.tensor_reduce(
                                m_wo_p[:], wot[:], axis=mybir.AxisListType.X,
                                op=Alu.max, apply_absolute_value=True)
                        else:
                            t2 = wk.tile([128, 1], dt.float32, tag="am_t",
                                         name="am_t")
                            nc.vector.tensor_reduce(
                                t2[:], wot[:], axis=mybir.AxisListType.X,
                                op=Alu.max, apply_absolute_value=True)
                            nc.vector.tensor_tensor(m_wo_p[:], m_wo_p[:], t2[:],
                                                    op=Alu.max)
                    m_wo = wk.tile([128, 1], dt.float32, tag="aa_wo", name="aa_wo")
                    nc.gpsimd.partition_all_reduce(
                        m_wo[:], m_wo_p[:], channels=128,
                        reduce_op=bass_isa.ReduceOp.absmax)

                    v8 = per0.tile([1, 8], dt.float32, tag="v8", name="v8")
                    nc.vector.memset(v8[:], 0.0)
                    for j, m in enumerate((m_x, m_wq, m_wk, m_wv, m_wo)):
                        nc.vector.tensor_copy(v8[:, j:j + 1], m[0:1, :])
                    nc.sync.dma_start(out=cc1_in.ap(), in_=v8[:])
                    nc.gpsimd.collective_compute(
                        "AllReduce", Alu.max, replica_groups=groups_all,
                        ins=[cc1_in.ap()], outs=[cc1_out.ap()])
                    g8 = per0.tile([1, 8], dt.float32, tag="g8", name="g8")
                    nc.sync.dma_start(out=g8[:], in_=cc1_out.ap())

                    s8 = per0.tile([1, 8], dt.float32, tag="s8", name="s8")
                    nc.vector.tensor_scalar(s8[:], g8[:], 1.0 / QMAX, EPS,
                                            op0=Alu.mult, op1=Alu.max)
                    inv8 = per0.tile([1, 8], dt.float32, tag="inv8", name="inv8")
                    nc.vector.reciprocal(inv8[:], s8[:])

                    inv_x = bc128(inv8[:, 0:1], "inv_x")
                    inv_q = bc128(inv8[:, 1:2], "inv_q")
                    inv_k = bc128(inv8[:, 2:3], "inv_k")
                    inv_v = bc128(inv8[:, 3:4], "inv_v")
                    inv_o = bc128(inv8[:, 4:5], "inv_o")

                    prod = per0.tile([1, 3], dt.float32, tag="prod", name="prod")
                    for j in range(3):
                        nc.vector.tensor_tensor(prod[:, j:j + 1], s8[:, 0:1],
                                                s8[:, j + 1:j + 2], op=Alu.mult)
                    sxwq = bc128(prod[:, 0:1], "sxwq")
                    sxwk = bc128(prod[:, 1:2], "sxwk")
                    sxwv = bc128(prod[:, 2:3], "sxwv")

                    xq = quantize(xf, inv_x, dt.bfloat16, "xq", xqp)
                    wqq = quantize(wfs["wq"], inv_q, dt.bfloat16, "wqq", xqp)
                    wkq = quantize(wfs["wk"], inv_k, dt.bfloat16, "wkq", xqp)
                    wvq = quantize(wfs["wv"], inv_v, dt.bfloat16, "wvq", xqp)

                # wo re-streamed + quantized (stage scope now closed)
                woq = []
                for i in range(KC_X):
                    wot = wk.tile([128, D], dt.float32, tag="wostg", name="wostg")
                    nc.sync.dma_start(out=wot[:], in_=woT[i * 128:(i + 1) * 128, :])
                    woq += quantize([wot], inv_o, dt.bfloat16, f"woq{i}_", woqp)

                # ---------------- Phase 3: q/k/v projections -------------
                with tc.tile_pool(name="qkvp", bufs=1) as qkvp:
                    qT, kT = [], []
                    for pname, wql, bt, sxw, dst in (
                            ("q", wqq, btiles["bq"], sxwq, qT),
                            ("k", wkq, btiles["bk"], sxwk, kT)):
                        for p in range(2):
                            qt_ = qkvp.tile([128, T], F32R,
                                            tag=f"{pname}T{p}", name=f"{pname}T{p}")
                            for tc_ in range(NQB):
                                ps = psp.tile([128, QB], dt.float32, tag="proj",
                                              name="proj_ps")
                                for kc in range(KC_X):
                                    nc.tensor.matmul(
                                        ps[:], wql[kc][:, p * 128:(p + 1) * 128],
                                        xq[kc][:, tc_ * QB:(tc_ + 1) * QB],
                                        start=(kc == 0), stop=(kc == KC_X - 1))
                                nc.vector.tensor_scalar(
                                    qt_[:, tc_ * QB:(tc_ + 1) * QB], ps[:],
                                    sxw[:], bt[:, p:p + 1],
                                    op0=Alu.mult, op1=Alu.add)
                            dst.append(qt_)

                    ones4 = qkvp.tile([128, HPC, 1], dt.float32, tag="ones4",
                                      name="ones4")
                    nc.vector.memset(ones4[:], 1.0)
                    v_sb = []
                    for tt in range(NT):
                        vt = qkvp.tile([128, HPC, HD + 1], F32R,
                                       tag=f"v{tt}", name=f"v{tt}")
                        nc.vector.tensor_copy(vt[:, :, HD:HD + 1], ones4[:])
                        ps = psp.tile([128, DPC], dt.float32, tag="proj",
                                      name="projv_ps")
                        for kc in range(KC_X):
                            nc.tensor.matmul(ps[:],
                                             xq[kc][:, tt * 128:(tt + 1) * 128],
                                             wvq[kc][:], start=(kc == 0),
                                             stop=(kc == KC_X - 1))
                        nc.vector.scalar_tensor_tensor(
                            vt[:, :, 0:HD],
                            ps[:].rearrange("p (h d) -> p h d", d=HD),
                            sxwv[:], bv_bc[:].rearrange("p (h d) -> p h d", d=HD),
                            op0=Alu.mult, op1=Alu.add)
                        v_sb.append(vt)

                    # ------------- Phase 4: attention ---------------------
                    with tc.tile_pool(name="attp", bufs=1) as attp:
                        masks = []
                        for r in range(4):
                            mt = attp.tile([128, 2 * QB], dt.float32,
                                           tag=f"mask{r}", name=f"mask{r}")
                            nc.sync.dma_start(out=mt[:], in_=masks_d[r])
                            masks.append(mt)

                        yT = [attp.tile([128, T], dt.float32, tag=f"yT{p}",
                                        name=f"yT{p}") for p in range(2)]
                        ymax = per0.tile([64, 1], dt.float32, tag="ymax",
                                         name="ymax")
                        nc.vector.memset(ymax[:], 0.0)

                        for p in range(2):          # head pair (dims tile)
                            for qb in range(NQB):
                                q0 = qb * QB
                                kmax = 4 * qb + 4
                                psA = psy.tile([HD + 1, QB], dt.float32,
                                               tag="psYA", name="psYA")
                                psB = psy.tile([HD + 1, QB], dt.float32,
                                               tag="psYB", name="psYB")
                                for kc in range(kmax):
                                    psS = psp.tile([128, 2 * QB], dt.float32,
                                                   tag="psS", name="psS")
                                    k0 = kc * 128
                                    nc.tensor.matmul(
                                        psS[:, 0:QB],
                                        kT[p][0:64, k0:k0 + 128],
                                        qT[p][0:64, q0:q0 + QB],
                                        start=True, stop=True,
                                        tile_position=(0, 0))
                                    nc.tensor.matmul(
                                        psS[:, QB:2 * QB],
                                        kT[p][64:128, k0:k0 + 128],
                                        qT[p][64:128, q0:q0 + QB],
                                        start=True, stop=True,
                                        tile_position=(64, 0))
                                    pt_ = ptp.tile([128, 2 * QB], F32R,
                                                   tag="PT", name="PT")
                                    nc.scalar.activation(pt_[:], psS[:], Act.Exp,
                                                         scale=SC)
                                    if kc >= 4 * qb:
                                        nc.vector.tensor_tensor(
                                            pt_[:], pt_[:],
                                            masks[kc - 4 * qb][:], op=Alu.mult)
                                    nc.tensor.matmul(
                                        psA[:],
                                        v_sb[kc][:, 2 * p, :],
                                        pt_[:, 0:QB],
                                        start=(kc == 0), stop=(kc == kmax - 1))
                                    nc.tensor.matmul(
                                        psB[:],
                                        v_sb[kc][:, 2 * p + 1, :],
                                        pt_[:, QB:2 * QB],
                                        start=(kc == 0), stop=(kc == kmax - 1))
                                for hl, psY in ((0, psA), (1, psB)):
                                    s1 = wk.tile([1, QB], dt.float32, tag="s1",
                                                 name="s1")
                                    nc.vector.reciprocal(s1[:], psY[64:65, :])
                                    rb = wk.tile([64, QB], dt.float32, tag="rb",
                                                 name="rb")
                                    nc.gpsimd.partition_broadcast(rb[:], s1[:])
                                    ysl = yT[p][hl * 64:(hl + 1) * 64,
                                                q0:q0 + QB]
                                    mt = wk.tile([64, 1], dt.float32, tag="ymt",
                                                 name="ymt")
                                    nc.vector.tensor_tensor(
                                        ysl, psY[0:64, :], rb[:], op=Alu.mult)
                                    nc.vector.tensor_reduce(
                                        mt[:], ysl, axis=mybir.AxisListType.X,
                                        op=Alu.max, apply_absolute_value=True)
                                    nc.vector.tensor_tensor(ymax[:], ymax[:],
                                                            mt[:], op=Alu.max)

                        # ------------- Phase 5: y scale (AllReduce) -------
                        yma = wk.tile([64, 1], dt.float32, tag="yma", name="yma")
                        nc.gpsimd.partition_all_reduce(
                            yma[:], ymax[:], channels=64,
                            reduce_op=bass_isa.ReduceOp.max)
                        v2 = per0.tile([1, 8], dt.float32, tag="v2", name="v2")
                        nc.vector.memset(v2[:], 0.0)
                        nc.vector.tensor_copy(v2[:, 0:1], yma[0:1, :])
                        nc.sync.dma_start(out=cc2_in.ap(), in_=v2[:])
                        nc.gpsimd.collective_compute(
                            "AllReduce", Alu.max, replica_groups=groups_all,
                            ins=[cc2_in.ap()], outs=[cc2_out.ap()])
                        g2 = per0.tile([1, 8], dt.float32, tag="g2", name="g2")
                        nc.sync.dma_start(out=g2[:], in_=cc2_out.ap())
                        sy = per0.tile([1, 1], dt.float32, tag="sy", name="sy")
                        nc.vector.tensor_scalar(sy[:], g2[:, 0:1], 1.0 / QMAX,
                                                EPS, op0=Alu.mult, op1=Alu.max)
                        inv_sy1 = per0.tile([1, 1], dt.float32, tag="inv_sy1",
                                            name="inv_sy1")
                        nc.vector.reciprocal(inv_sy1[:], sy[:])
                        inv_sy = bc128(inv_sy1[:], "inv_sy")
                        sywo1 = per0.tile([1, 1], dt.float32, tag="sywo1",
                                          name="sywo1")
                        nc.vector.tensor_tensor(sywo1[:], sy[:], s8[:, 4:5],
                                                op=Alu.mult)
                        sywo = bc128(sywo1[:], "sywo")

                        # ------------- Phase 6: quantize y, AllGather -----
                        with tc.tile_pool(name="latep", bufs=1) as latep:
                            yq = quantize(yT, inv_sy, dt.int8, "yq", latep)
                            for p in range(2):
                                nc.sync.dma_start(
                                    out=ag_in.ap()[p * 128:(p + 1) * 128, :],
                                    in_=yq[p][:])
                            nc.gpsimd.collective_compute(
                                "AllGather", Alu.bypass, replica_groups=groups_b,
                                ins=[ag_in.ap()], outs=[ag_out.ap()])
                            treg = nc.alloc_registers()
                            nc.regs_load(treg, toff_d[0:1, 0:1])
                            toff = nc.snap(treg, donate=True, min_val=0,
                                           max_val=T - QB)
                            yg = []
                            for i in range(KC_X):
                                gi = wk.tile([128, QB], dt.int8, tag="ygi",
                                             name="ygi")
                                nc.sync.dma_start(
                                    out=gi[:],
                                    in_=ag_out.ap()[i * 128:(i + 1) * 128,
                                                    bass.ds(toff, QB)])
                                gb = latep.tile([128, QB], dt.bfloat16,
                                                tag=f"ygb{i}", name=f"ygb{i}")
                                nc.vector.tensor_copy(gb[:], gi[:])
                                yg.append(gb)

                            # --------- Phase 7: output projection ---------
                            for tt in range(TPC // 128):
                                ot = latep.tile([128, D], dt.float32, tag="ot",
                                                name="ot")
                                for oc in range(2):
                                    ps = psp.tile([128, QB], dt.float32,
                                                  tag="proj", name="proj_o")
                                    for kc in range(KC_X):
                                        nc.tensor.matmul(
                                            ps[:],
                                            yg[kc][:, tt * 128:(tt + 1) * 128],
                                            woq[kc][:, oc * QB:(oc + 1) * QB],
                                            start=(kc == 0),
                                            stop=(kc == KC_X - 1))
                                    nc.vector.scalar_tensor_tensor(
                                        ot[:, oc * QB:(oc + 1) * QB], ps[:],
                                        sywo[:], bo_bc[:, oc * QB:(oc + 1) * QB],
                                        op0=Alu.mult, op1=Alu.add)
                                nc.sync.dma_start(
                                    out=out_d[tt * 128:(tt + 1) * 128, :],
                                    in_=ot[:])

    nc.compile()
    return nc


def _masks_np():
    """masks[r][k, q] = 1 if q >= k + 128*r else 0, duplicated for both heads."""
    m = np.zeros((4, 128, 2 * QB), np.float32)
    q = np.arange(QB)[None, :]
    k = np.arange(128)[:, None]
    for r in range(4):
        mm = (q >= k + 128 * r).astype(np.float32)
        m[r, :, 0:QB] = mm
        m[r, :, QB:2 * QB] = mm
    return m


def make_in_maps(x, Wq, bq, Wk, bk, Wv, bv, Wo, bo):
    masks = _masks_np()
    woT = np.ascontiguousarray(Wo.T)
    xTs = [np.ascontiguousarray(x[b].T) for b in range(B)]
    wT = {}
    for name, W in (("q", Wq), ("k", Wk), ("v", Wv)):
        wT[name] = [np.ascontiguousarray(W[g * DPC:(g + 1) * DPC, :].T)
                    for g in range(GROUP)]
    in_maps = []
    for c in range(NCORES):
        b, hg = c // GROUP, c % GROUP
        in_maps.append({
            "xT": xTs[b],
            "wqT": wT["q"][hg], "wkT": wT["k"][hg], "wvT": wT["v"][hg],
            "woT": woT,
            "bq": np.ascontiguousarray(bq[hg * DPC:(hg + 1) * DPC]),
            "bk": np.ascontiguousarray(bk[hg * DPC:(hg + 1) * DPC]),
            "bv": np.ascontiguousarray(bv[hg * DPC:(hg + 1) * DPC]),
            "bo": bo,
            "masks": masks,
            "toff": np.array([[hg * QB]], dtype=np.uint32),
        })
    return in_maps


def assemble(results):
    out = np.empty((B, T, D), np.float32)
    for c in range(NCORES):
        b, hg = c // GROUP, c % GROUP
        out[b, hg * TPC:(hg + 1) * TPC, :] = results[c]["out"]
    return out


def kernel(x, Wq, bq, Wk, bk, Wv, bv, Wo, bo):
    if "nc" not in _CACHE:
        _CACHE["nc"] = build()
    nc = _CACHE["nc"]
    in_maps = make_in_maps(x, Wq, bq, Wk, bk, Wv, bv, Wo, bo)
    res = run_bass_kernel_spmd(nc, in_maps, list(range(NCORES)))
    return assemble(res.results)
